# revision 1
# baseline (speedup 1.0000x reference)
"""Chunked cross-attention TRN2 kernel (8 NeuronCores, SPMD).

Problem (hardcoded): B=4, S=2048, HIDDEN=1024, heads=16, head_dim=64,
chunks C=32 x chunk_len 64, neighbors N=2 x L=128 (256 keys per chunk).

Sharding: the B*C = 128 (batch, chunk) pairs are split evenly across the 8
cores (16 pairs each). Each core projects Q/K/V for its pairs, runs the
chunk-local softmax attention, and writes its pairs' outputs. Weights are
replicated per core. No collectives needed.

Numerics: projection matmuls run as float32r (TF32-class, full PE rate at
free-dim >= 256); the attention matmuls (scores/transpose/AV) run in bf16,
which legalizes tile_position head-packing. Accumulation is always fp32 in
PSUM. Softmax runs without max-subtraction (shift-invariant, and
|scores/8| is small for randn-scale inputs so exp cannot overflow); the
exp is computed by ScalarE with a fused per-row sum, and the 1/sum is
applied on the AV result during the PSUM->SBUF copyback.

Layouts (host-prepared so the device never transposes activations):
  q_t   [1024, 16*64]   f32r  shifted/padded query, hidden-major
  kv_t  [1024, 16*256]  f32r  kv rows, hidden-major
  wq_t  [8*128*8*128]   f32r  W_q.T prepacked per m-tile [mo, p, ko, m]
  wk_t, wv_t [1024,1024] f32r W.T (contraction-major)
  bq_t/bk_t [128, 8]    f32   bias striped per m-subtile
  bv_r  [128, 1024]     f32   V bias replicated across partitions
Device out [16, 128, 512] f32: partitions = 2 heads x 64 rows (head-pair
packing via tile_position quadrants), free = head_pair*64 + d. The host
unpacks this and applies the chunked-attention output shift.
"""

import numpy as np

B, S, HID = 4, 2048, 1024
C, NNB, L = 32, 2, 128
CHUNK = 64
NHEADS, HEAD = 16, 64
NCORES = 8
NPAIRS = B * C                 # 128
PER_CORE = NPAIRS // NCORES    # 16
QBLK = 4                       # pairs per Q-projection block (rows = 256)
GRP = 2                        # pairs per K/V-projection group (rows = 512)
P = 128
KSUB = HID // P                # 8
MSUB = HID // P                # 8
JTOT = NNB * L                 # 256 keys per chunk
SCALE = 0.125                  # 1/sqrt(HEAD)

_CACHE = {}


def _build():
    from contextlib import ExitStack

    import concourse.bass as bass
    import concourse.mybir as mybir
    import concourse.tile as tile
    from concourse import bacc
    from concourse.masks import make_identity

    f32 = mybir.dt.float32
    f32r = mybir.dt.float32r
    bf16 = mybir.dt.bfloat16

    nc = bacc.Bacc("TRN2", target_bir_lowering=False, debug=False,
                   num_devices=NCORES)

    q_t = nc.dram_tensor("q_t", [HID, PER_CORE * CHUNK], f32r, kind="ExternalInput")
    kv_t = nc.dram_tensor("kv_t", [HID, PER_CORE * JTOT], f32r, kind="ExternalInput")
    wq_t = nc.dram_tensor("wq_t", [MSUB * P * KSUB * P], f32r, kind="ExternalInput")
    wk_t = nc.dram_tensor("wk_t", [HID, HID], f32r, kind="ExternalInput")
    wv_t = nc.dram_tensor("wv_t", [HID, HID], f32r, kind="ExternalInput")
    bq_t = nc.dram_tensor("bq_t", [P, MSUB], f32, kind="ExternalInput")
    bk_t = nc.dram_tensor("bk_t", [P, MSUB], f32, kind="ExternalInput")
    bv_r = nc.dram_tensor("bv_r", [P, HID], f32, kind="ExternalInput")
    out = nc.dram_tensor("out", [PER_CORE, P, NHEADS * HEAD // 2], f32,
                         kind="ExternalOutput")

    q_td = q_t[:].rearrange("(ko p) r -> p ko r", p=P)
    kv_td = kv_t[:].rearrange("(ko p) r -> p ko r", p=P)
    wq_packed = wq_t[:].rearrange("(mo p ko m) -> mo p ko m",
                                  mo=MSUB, p=P, ko=KSUB)
    wk_td = wk_t[:].rearrange("(ko p) m -> p ko m", p=P)
    wv_td = wv_t[:].rearrange("(ko p) m -> p ko m", p=P)

    with tile.TileContext(nc) as tc:
        with ExitStack() as ctx:
            wpool = ctx.enter_context(tc.tile_pool(name="weights", bufs=1))
            qtp = ctx.enter_context(tc.tile_pool(name="qt", bufs=1))
            qpp = ctx.enter_context(tc.tile_pool(name="qproj", bufs=2))
            kvp = ctx.enter_context(tc.tile_pool(name="kvt", bufs=2))
            kpp = ctx.enter_context(tc.tile_pool(name="kproj", bufs=3))
            vpp = ctx.enter_context(tc.tile_pool(name="vproj", bufs=3))
            sfp = ctx.enter_context(tc.tile_pool(name="soft", bufs=3))
            smalls = ctx.enter_context(tc.tile_pool(name="smalls", bufs=6))
            outp = ctx.enter_context(tc.tile_pool(name="outsb", bufs=2))
            ps_pj = ctx.enter_context(tc.tile_pool(name="ps_pj", bufs=3, space="PSUM"))
            ps_sc = ctx.enter_context(tc.tile_pool(name="ps_sc", bufs=2, space="PSUM"))
            ps_tr = ctx.enter_context(tc.tile_pool(name="ps_tr", bufs=1, space="PSUM"))
            ps_av = ctx.enter_context(tc.tile_pool(name="ps_av", bufs=2, space="PSUM"))

            # --- resident constants. DMA order minimizes bytes before the
            # first matmul (wq m-tile 0 + q block 0) and before the first
            # K-projection (wk + first kv tiles ahead of wv). ---
            bq_sb = wpool.tile([P, MSUB], f32)
            nc.sync.dma_start(bq_sb[:], bq_t[:])
            wq_ts = []
            for mo in range(MSUB):
                w = wpool.tile([P, KSUB, P], f32r, name=f"wq{mo}")
                nc.sync.dma_start(w[:], wq_packed[mo])
                wq_ts.append(w)
                if mo == 0:
                    qt_first = qtp.tile([P, KSUB, QBLK * CHUNK], f32r,
                                        tag="qt", name="qt_first")
                    nc.sync.dma_start(qt_first[:],
                                      q_td[:, :, bass.ts(0, QBLK * CHUNK)])
            bk_sb = wpool.tile([P, MSUB], f32)
            nc.sync.dma_start(bk_sb[:], bk_t[:])
            wk_sb = wpool.tile([P, KSUB, HID], f32r)
            nc.sync.dma_start(wk_sb[:], wk_td)
            # block 0's kv tiles load before wv so the K projection can
            # start as soon as the Q projection drains
            kvt_first = []
            for g2 in range(QBLK // GRP):
                kvt_sb = kvp.tile([P, KSUB, GRP * JTOT], f32r,
                                  tag="kvt", name=f"kvt_first{g2}")
                nc.sync.dma_start(
                    kvt_sb[:], kv_td[:, :, bass.ds(g2 * GRP * JTOT, GRP * JTOT)])
                kvt_first.append(kvt_sb)
            wv_sb = wpool.tile([P, KSUB, HID], f32r)
            nc.sync.dma_start(wv_sb[:], wv_td)
            bv_sb = wpool.tile([P, HID], f32)
            nc.sync.dma_start(bv_sb[:], bv_r[:])
            ident32 = wpool.tile([P, P], f32)
            make_identity(nc, ident32[:])
            ident = wpool.tile([P, P], bf16)
            nc.vector.tensor_copy(ident[:], ident32[:])

            Exp = mybir.ActivationFunctionType.Exp
            Ident = mybir.ActivationFunctionType.Identity

            for blk in range(PER_CORE // QBLK):
                # ---- Q projection for this block (rows = QBLK*64 = 256) ----
                if blk == 0:
                    qt_sb = qt_first
                else:
                    qt_sb = qtp.tile([P, KSUB, QBLK * CHUNK], f32r, tag="qt",
                                     name="qt_sb")
                    nc.sync.dma_start(qt_sb[:],
                                      q_td[:, :, bass.ts(blk, QBLK * CHUNK)])
                qp_sb = qpp.tile([P, MSUB, QBLK * CHUNK], bf16)
                for mo in range(MSUB):
                    pt = ps_pj.tile([P, 512], f32, tag="ps_pj", name="pt")
                    pt = pt[:, : QBLK * CHUNK]
                    for k in range(KSUB):
                        nc.tensor.matmul(
                            pt[:],
                            wq_ts[mo][:, k, :],
                            qt_sb[:, k, :],
                            start=(k == 0),
                            stop=(k == KSUB - 1),
                        )
                    nc.scalar.activation(qp_sb[:, mo, :], pt[:], Ident,
                                         bias=bq_sb[:, mo, None])

                # ---- K/V projections, 2 groups of 512 kv rows each ----
                if blk == 0:
                    kvt_ts = kvt_first
                else:
                    kvt_ts = []
                    for g2 in range(QBLK // GRP):
                        kvt_sb = kvp.tile([P, KSUB, GRP * JTOT], f32r,
                                          tag="kvt", name=f"kvt{g2}")
                        nc.sync.dma_start(
                            kvt_sb[:],
                            kv_td[:, :, bass.ds((blk * QBLK + g2 * GRP) * JTOT,
                                                GRP * JTOT)])
                        kvt_ts.append(kvt_sb)

                kp_ts = [kpp.tile([P, MSUB, GRP * JTOT], bf16, tag="kp",
                                  name=f"kp{g2}")
                         for g2 in range(QBLK // GRP)]
                for g2 in range(QBLK // GRP):
                    for mo in range(MSUB):
                        pt = ps_pj.tile([P, 512], f32, tag="ps_pj", name="pt")
                        for k in range(KSUB):
                            nc.tensor.matmul(
                                pt[:],
                                wk_sb[:, k, bass.ts(mo, P)],
                                kvt_ts[g2][:, k, :],
                                start=(k == 0),
                                stop=(k == KSUB - 1),
                            )
                        nc.scalar.activation(kp_ts[g2][:, mo, :], pt[:],
                                             Ident, bias=bk_sb[:, mo, None])

                vp_ts = [vpp.tile([P, 2 * GRP, HID], bf16, tag="vp",
                                  name=f"vp{g2}")
                         for g2 in range(QBLK // GRP)]
                for g2 in range(QBLK // GRP):
                    for rt in range(2 * GRP):
                        for nt in range(2):
                            pt = ps_pj.tile([P, 512], f32, tag="ps_pj",
                                            name="pt")
                            for k in range(KSUB):
                                nc.tensor.matmul(
                                    pt[:],
                                    kvt_ts[g2][:, k, bass.ts(rt, P)],
                                    wv_sb[:, k, bass.ts(nt, 512)],
                                    start=(k == 0),
                                    stop=(k == KSUB - 1),
                                )
                            nc.vector.tensor_tensor(
                                vp_ts[g2][:, rt, bass.ts(nt, 512)], pt[:],
                                bv_sb[:, bass.ts(nt, 512)],
                                mybir.AluOpType.add,
                            )

                # ---- attention: two heads packed per 128 partitions ----
                for pi in range(QBLK):
                    g2, g = pi // GRP, pi % GRP
                    kp_sb, vp_sb = kp_ts[g2], vp_ts[g2]
                    gp = blk * QBLK + pi
                    out_sb = outp.tile([P, NHEADS * HEAD // 2], f32,
                                       tag="out_sb")
                    for hp in range(NHEADS // 2):
                        ps_s = ps_sc.tile([P, JTOT], f32, tag="ps_s")
                        nc.tensor.matmul(
                            ps_s[0:64, :],
                            qp_sb[0:64, hp, bass.ts(pi, CHUNK)],
                            kp_sb[0:64, hp, bass.ts(g, JTOT)],
                            start=True, stop=True, tile_position=(0, 0),
                        )
                        nc.tensor.matmul(
                            ps_s[64:128, :],
                            qp_sb[64:128, hp, bass.ts(pi, CHUNK)],
                            kp_sb[64:128, hp, bass.ts(g, JTOT)],
                            start=True, stop=True, tile_position=(64, 64),
                        )
                        attn = sfp.tile([P, JTOT], bf16, tag="attn")
                        rsum = smalls.tile([P, 1], f32, tag="rsum")
                        nc.scalar.activation(attn[:], ps_s[:], Exp,
                                             scale=SCALE, accum_out=rsum[:])
                        recip = smalls.tile([P, 1], f32, tag="recip")
                        nc.vector.reciprocal(recip[:], rsum[:])

                        ps_t = ps_tr.tile([P, 2, P], bf16, tag="ps_t")
                        for jh in range(2):
                            nc.tensor.transpose(
                                ps_t[:, jh, :], attn[:, bass.ts(jh, P)],
                                ident)
                        at_t = sfp.tile([P, 2, P], bf16, tag="at_t")
                        nc.vector.tensor_copy(at_t[:], ps_t[:])

                        ps_o = ps_av.tile([P, HEAD], f32, tag="ps_o")
                        for jh in range(2):
                            nc.tensor.matmul(
                                ps_o[0:64, :],
                                at_t[:, jh, 0:64],
                                vp_sb[:, 2 * g + jh,
                                      bass.ds(2 * hp * HEAD, HEAD)],
                                start=(jh == 0), stop=(jh == 1),
                                tile_position=(0, 0),
                            )
                            nc.tensor.matmul(
                                ps_o[64:128, :],
                                at_t[:, jh, 64:128],
                                vp_sb[:, 2 * g + jh,
                                      bass.ds((2 * hp + 1) * HEAD, HEAD)],
                                start=(jh == 0), stop=(jh == 1),
                                tile_position=(0, 64),
                            )
                        nc.vector.tensor_scalar_mul(
                            out_sb[:, bass.ts(hp, HEAD)], ps_o[:], recip[:])

                    nc.sync.dma_start(out[gp], out_sb[:])

    nc.finalize()
    return nc


def _prepare_inputs(query, kv, Wq, bq, Wk, bk, Wv, bv):
    """Build the 8 per-core input maps (host-side shard + layout + cast)."""
    import ml_dtypes

    f32 = np.float32
    bf = ml_dtypes.bfloat16
    query = np.asarray(query, dtype=f32)
    kv = np.asarray(kv, dtype=f32)

    # shift right by CHUNK-1, pad to C*CHUNK rows
    q_shift = np.zeros((B, C * CHUNK, HID), dtype=f32)
    q_shift[:, : S - (CHUNK - 1)] = query[:, CHUNK - 1:]
    q_pairs = q_shift.reshape(B * C, CHUNK, HID)
    kv_pairs = kv.reshape(B * C, JTOT, HID)

    wq_tt = np.asarray(Wq, dtype=f32).T  # [h, m]
    wq_t = np.ascontiguousarray(
        wq_tt.reshape(KSUB, P, MSUB, P).transpose(2, 1, 0, 3)
    ).reshape(-1)
    wk_t = np.ascontiguousarray(np.asarray(Wk, dtype=f32).T)
    wv_t = np.ascontiguousarray(np.asarray(Wv, dtype=f32).T)
    bq_t = np.ascontiguousarray(np.asarray(bq, dtype=f32).reshape(MSUB, P).T)
    bk_t = np.ascontiguousarray(np.asarray(bk, dtype=f32).reshape(MSUB, P).T)
    bv_rep = np.ascontiguousarray(
        np.broadcast_to(np.asarray(bv, dtype=f32), (P, HID)))

    in_maps = []
    for ci in range(NCORES):
        sel = slice(ci * PER_CORE, (ci + 1) * PER_CORE)
        q_core = q_pairs[sel].reshape(PER_CORE * CHUNK, HID)
        kv_core = kv_pairs[sel].reshape(PER_CORE * JTOT, HID)
        in_maps.append({
            "q_t": np.ascontiguousarray(q_core.T),
            "kv_t": np.ascontiguousarray(kv_core.T),
            "wq_t": wq_t,
            "wk_t": wk_t,
            "wv_t": wv_t,
            "bq_t": bq_t,
            "bk_t": bk_t,
            "bv_r": bv_rep,
        })
    return in_maps


def _unpack_output(results):
    """results: list of 8 dicts with 'out' [16, 128, 512] -> full (B,S,HID)."""
    h = np.empty((NPAIRS, CHUNK, HID), dtype=np.float32)
    for ci in range(NCORES):
        arr = results[ci]["out"]
        a = arr.reshape(PER_CORE, 2, CHUNK, NHEADS // 2, HEAD)
        a = a.transpose(0, 2, 3, 1, 4).reshape(PER_CORE, CHUNK, HID)
        h[ci * PER_CORE:(ci + 1) * PER_CORE] = a
    h = h.reshape(B, C * CHUNK, HID)
    outp = np.zeros((B, S, HID), dtype=np.float32)
    outp[:, CHUNK - 1:] = h[:, : S - (CHUNK - 1)]
    return outp


def kernel(query, kv, Wq, bq, Wk, bk, Wv, bv):
    from concourse.bass_utils import run_bass_kernel_spmd

    if "nc" not in _CACHE:
        _CACHE["nc"] = _build()
    nc = _CACHE["nc"]

    in_maps = _prepare_inputs(query, kv, Wq, bq, Wk, bk, Wv, bv)
    res = run_bass_kernel_spmd(nc, in_maps, list(range(NCORES)))
    return _unpack_output(res.results)

